# revision 1
# baseline (speedup 1.0000x reference)
# Trainium2 Bass kernel for the Tacotron-style decoder (2-layer LSTM, B=32,
# T=1000). Strategy: data-parallel over batch, 4 sequences per NeuronCore.
# All compute is local per core (no collectives):
#   Ph1  transpose memory + shifted mels to channel-major
#   Ph2  prenet (2x matmul+relu), channel-major
#   Ph3  xg0 = w_ih0 @ x + b   (batched over all timesteps)
#   Ph4  layer-0 LSTM recurrence (weights stream through PE as bf16 stationary)
#   Ph5  xg1 = w_ih1 @ h0 + b  (batched)
#   Ph6  layer-1 LSTM recurrence
#   Ph7  projection out = W_proj @ [h1; mem] + b
# Gates are kept channel-major [128ch, (i|f|o|g) x 4batch] so the elementwise
# LSTM cell runs on [128, 4..16] tiles and hides under the PE weight stream.
import functools
import numpy as np
import ml_dtypes

B, T, A, M = 32, 1000, 512, 80
P, H = 256, 1024
NCORES = 8
BC = B // NCORES            # 4 sequences per core
F = BC * T                  # 4000 frames per core, frame f = t*BC + b
G4 = 4 * H                  # 4096 gate rows
NBLK = H // 128             # 8 channel blocks
SBLK = 8                    # recurrence steps per hardware-loop iteration
# gate order used on-chip: i, f, o, g  (PyTorch order is i, f, g, o)
GORDER = (0, 1, 3, 2)
NCHUNK = 8                  # frame chunks for batched GEMMs
FCH = F // NCHUNK           # 500 frames per chunk


def _arrange_cols(wt):
    """wt [K, 4096] (= w.T, PyTorch gate order i,f,g,o on columns) ->
    columns reordered to m-index = blk*4 + gi with gi over (i,f,o,g)."""
    cols = []
    for blk in range(NBLK):
        for go in GORDER:
            cols.append(wt[:, go * H + blk * 128: go * H + (blk + 1) * 128])
    return np.ascontiguousarray(np.concatenate(cols, axis=1))


def _arrange_vec(b):
    return _arrange_cols(b.reshape(1, G4))[0]


@functools.lru_cache(maxsize=1)
def _build():
    import concourse.bacc as bacc
    import concourse.mybir as mybir
    from concourse import tile

    dt = mybir.dt
    nc = bacc.Bacc(None)

    # ---------------- I/O ----------------
    mem_f = nc.declare_dram_parameter("mem_f", [F, A], dt.float32, isOutput=False)
    y_f = nc.declare_dram_parameter("y_f", [F, M], dt.float32, isOutput=False)
    ident = nc.declare_dram_parameter("ident", [128, 128], dt.float32, isOutput=False)
    w1t = nc.declare_dram_parameter("w1t", [M, P], dt.float32, isOutput=False)
    w2t = nc.declare_dram_parameter("w2t", [P, P], dt.float32, isOutput=False)
    wih0t = nc.declare_dram_parameter("wih0t", [P + A, G4], dt.float32, isOutput=False)
    whh0t = nc.declare_dram_parameter("whh0t", [H, G4], dt.bfloat16, isOutput=False)
    wih1t = nc.declare_dram_parameter("wih1t", [H, G4], dt.bfloat16, isOutput=False)
    whh1t = nc.declare_dram_parameter("whh1t", [H, G4], dt.bfloat16, isOutput=False)
    b0in = nc.declare_dram_parameter("b0in", [1, G4], dt.float32, isOutput=False)
    b1in = nc.declare_dram_parameter("b1in", [1, G4], dt.float32, isOutput=False)
    wpt_h = nc.declare_dram_parameter("wpt_h", [H, M], dt.bfloat16, isOutput=False)
    wpt_m = nc.declare_dram_parameter("wpt_m", [A, M], dt.float32, isOutput=False)
    bpin = nc.declare_dram_parameter("bpin", [1, M], dt.float32, isOutput=False)
    outT = nc.declare_dram_parameter("outT", [M, F], dt.float32, isOutput=True)

    # ---------------- internal DRAM ----------------
    memT_d = nc.dram_tensor("memT_d", [A, F], dt.float32)
    xg0T = nc.dram_tensor("xg0T", [G4, F], dt.float32)
    h0T = nc.dram_tensor("h0T", [H, F], dt.bfloat16)
    xg1T = nc.dram_tensor("xg1T", [G4, F], dt.float32)
    h1T = nc.dram_tensor("h1T", [H, F], dt.bfloat16)

    FT = (F + 127) // 128  # 32 frame tiles (31 full + 1 of 32 rows)

    def ftrows(ft):
        return min(128, F - ft * 128)

    ACT = mybir.ActivationFunctionType

    with tile.TileContext(nc) as tc:
        with tc.tile_pool(name="const", bufs=1) as cpool:
            idsb = cpool.tile([128, 128], dt.float32, name="idsb")
            nc.sync.dma_start(idsb[:], ident[:])
            b0sb = cpool.tile([128, 32], dt.float32, name="b0sb")
            b1sb = cpool.tile([128, 32], dt.float32, name="b1sb")
            bpsb = cpool.tile([M, 1], dt.float32, name="bpsb")
            # bias column m at b*sb[:, m]
            nc.sync.dma_start(b0sb[:], b0in[:].rearrange("o (m p) -> (o p) m", p=128))
            nc.sync.dma_start(b1sb[:], b1in[:].rearrange("o (m p) -> (o p) m", p=128))
            nc.sync.dma_start(bpsb[:], bpin[:].rearrange("o (m u) -> (o m) u", u=1))

            # persistent channel-major activations
            with tc.tile_pool(name="actsb", bufs=1) as apool:
                prevT = apool.tile([M, F], dt.float32, name="prevT")
                p2T = apool.tile([128, 2 * F], dt.float32, name="p2T")

                # ---------- Ph1: transposes ----------
                with tc.tile_pool(name="tr", bufs=3) as trp, \
                     tc.tile_pool(name="trps", bufs=2, space="PSUM") as trps:
                    for ft in range(FT):
                        r = ftrows(ft)
                        # shifted mels -> prevT
                        yin = trp.tile([128, M], dt.float32, name="yin", tag="yin")
                        if ft == 0:
                            nc.gpsimd.memset(yin[:, :], 0.0)
                            nc.sync.dma_start(yin[BC:r, :], y_f[0:r - BC, :])
                        else:
                            nc.sync.dma_start(yin[0:r, :], y_f[ft * 128 - BC: ft * 128 - BC + r, :])
                        yps = trps.tile([M, 128], dt.float32, name="yps", tag="yps")
                        nc.tensor.transpose(yps[:, 0:r], yin[0:r, :], idsb[0:r, 0:r])
                        nc.scalar.copy(prevT[:, ft * 128: ft * 128 + r], yps[:, 0:r])
                        # memory -> memT (4 column blocks)
                        for cb in range(A // 128):
                            min_ = trp.tile([128, 128], dt.float32, name="min_", tag="min")
                            nc.sync.dma_start(min_[0:r, :], mem_f[ft * 128: ft * 128 + r, cb * 128:(cb + 1) * 128])
                            mps = trps.tile([128, 128], dt.float32, name="mps", tag="mps")
                            nc.tensor.transpose(mps[:, 0:r], min_[0:r, :], idsb[0:r, 0:r])
                            mrow = trp.tile([128, 128], dt.float32, name="mrow", tag="mrow")
                            nc.scalar.copy(mrow[:, 0:r], mps[:, 0:r])
                            nc.sync.dma_start(memT_d[cb * 128:(cb + 1) * 128, ft * 128: ft * 128 + r], mrow[:, 0:r])

                # ---------- Ph2: prenet ----------
                with tc.tile_pool(name="pn", bufs=2) as pnp, \
                     tc.tile_pool(name="pnps", bufs=2, space="PSUM") as pnps:
                    w1sb = pnp.tile([M, P], dt.float32, name="w1sb")
                    nc.sync.dma_start(w1sb[:], w1t[:])
                    p1T = pnp.tile([128, 2 * F], dt.float32, name="p1T")
                    for m in range(P // 128):
                        for n in range(NCHUNK):
                            ps = pnps.tile([128, FCH], dt.float32, name="pnps1", tag=f"pn{n % 4}")
                            nc.tensor.matmul(ps[:], w1sb[:, m * 128:(m + 1) * 128],
                                             prevT[:, n * FCH:(n + 1) * FCH], start=True, stop=True)
                            nc.scalar.activation(p1T[:, m * F + n * FCH: m * F + (n + 1) * FCH], ps[:], ACT.Relu)
                    w2sb = pnp.tile([128, 2 * P], dt.float32, name="w2sb")
                    for k in range(P // 128):
                        nc.sync.dma_start(w2sb[:, k * P:(k + 1) * P], w2t[k * 128:(k + 1) * 128, :])
                    for m in range(P // 128):
                        for n in range(NCHUNK):
                            ps = pnps.tile([128, FCH], dt.float32, name="pnps2", tag=f"pn{n % 4}")
                            for k in range(P // 128):
                                nc.tensor.matmul(ps[:], w2sb[:, k * P + m * 128: k * P + (m + 1) * 128],
                                                 p1T[:, k * F + n * FCH: k * F + (n + 1) * FCH],
                                                 start=(k == 0), stop=(k == 1))
                            nc.scalar.activation(p2T[:, m * F + n * FCH: m * F + (n + 1) * FCH], ps[:], ACT.Relu)

                # ---------- Ph3: xg0 ----------
                # rhs K-tiles: 2 from p2T, 4 from memT (SBUF-resident copy)
                KX = 6
                with tc.tile_pool(name="x0", bufs=2) as x0p, \
                     tc.tile_pool(name="x0ps", bufs=1, space="PSUM") as x0ps:
                    memTsb = x0p.tile([128, 4 * F], dt.float32, name="memTsb")
                    for cb in range(4):
                        nc.sync.dma_start(memTsb[:, cb * F:(cb + 1) * F], memT_d[cb * 128:(cb + 1) * 128, :])

                    def x_rhs(k, n):
                        if k < 2:
                            return p2T[:, k * F + n * FCH: k * F + (n + 1) * FCH]
                        cb = k - 2
                        return memTsb[:, cb * F + n * FCH: cb * F + n * FCH + FCH]

                    for m in range(32):
                        wtile = x0p.tile([128, 6 * 128], dt.float32, name="wtile", tag="w0t")
                        for k in range(KX):
                            nc.sync.dma_start(
                                wtile[:, k * 128:(k + 1) * 128],
                                wih0t[k * 128:(k + 1) * 128, m * 128:(m + 1) * 128])
                        pss = []
                        for n in range(NCHUNK):
                            ps = x0ps.tile([128, FCH], dt.float32, name="x0psn", tag=f"x0{n}")
                            pss.append(ps)
                        for k in range(KX):
                            for n in range(NCHUNK):
                                nc.tensor.matmul(pss[n][:], wtile[:, k * 128:(k + 1) * 128], x_rhs(k, n),
                                                 start=(k == 0), stop=(k == KX - 1))
                        for n in range(NCHUNK):
                            otile = x0p.tile([128, FCH], dt.float32, name="otile", tag="x0o")
                            nc.vector.tensor_scalar_add(otile[:], pss[n][:], b0sb[:, m:m + 1])
                            nc.sync.dma_start(xg0T[m * 128:(m + 1) * 128, n * FCH:(n + 1) * FCH], otile[:])

            # ---------- recurrence helper ----------
            def recurrence(whhT_in, xgT_d, hT_out):
                NB = T // SBLK  # 125 blocks
                with tc.tile_pool(name="rc", bufs=1) as rp, \
                     tc.tile_pool(name="rcx", bufs=2) as rxp, \
                     tc.tile_pool(name="rcps", bufs=1, space="PSUM") as rps, \
                     tc.tile_pool(name="rct", bufs=2) as rtp:
                    whsb = rp.tile([128, 8 * G4], dt.bfloat16, name="whsb")
                    for k in range(8):
                        nc.sync.dma_start(whsb[:, k * G4:(k + 1) * G4], whhT_in[k * 128:(k + 1) * 128, :])
                    hbuf = [rp.tile([128, 4 * NBLK], dt.bfloat16, name=f"hbuf{i}") for i in range(2)]
                    cbuf = [rp.tile([128, 4 * NBLK], dt.float32, name=f"cbuf{i}") for i in range(2)]
                    nc.gpsimd.memset(hbuf[0][:], 0.0)
                    nc.gpsimd.memset(cbuf[0][:], 0.0)
                    psl = [rps.tile([128, 16], dt.float32, name=f"ps{blk}", tag=f"ps{blk}") for blk in range(NBLK)]

                    import concourse.bass as bass

                    with tc.For_i(0, NB, 1, hint_engines=(mybir.EngineType.PE,
                                                          mybir.EngineType.DVE,
                                                          mybir.EngineType.Activation)) as bi:
                        xgsb = rxp.tile([128, 32 * 4 * SBLK], dt.float32, name="xgsb", tag="xgsb")
                        for rr in range(32):
                            nc.sync.dma_start(
                                xgsb[:, rr * 4 * SBLK:(rr + 1) * 4 * SBLK],
                                xgT_d[rr * 128:(rr + 1) * 128, bass.ts(bi, 4 * SBLK)])
                        hblk = rxp.tile([128, NBLK * 4 * SBLK], dt.bfloat16, name="hblk", tag="hblk")
                        for s in range(SBLK):
                            pin, pout = s % 2, 1 - (s % 2)
                            h_in, h_out = hbuf[pin], hbuf[pout]
                            c_in, c_out = cbuf[pin], cbuf[pout]
                            for blk in range(NBLK):
                                ps = psl[blk]
                                for gi in range(4):
                                    mm = blk * 4 + gi
                                    for k in range(8):
                                        nc.tensor.matmul(
                                            ps[:, gi * 4:(gi + 1) * 4],
                                            whsb[:, k * G4 + mm * 128: k * G4 + (mm + 1) * 128],
                                            h_in[:, k * 4:(k + 1) * 4],
                                            start=(k == 0), stop=(k == 7))
                                # elementwise cell for this channel block
                                zt = rtp.tile([128, 16], dt.float32, name="zt", tag=f"zt{blk % 4}")
                                xga = xgsb[:].rearrange("p (r c) -> p r c", r=32)[
                                    :, blk * 4: blk * 4 + 4, s * 4: s * 4 + 4]
                                psa = ps[:].rearrange("p (r c) -> p r c", r=4)
                                zta = zt[:].rearrange("p (r c) -> p r c", r=4)
                                nc.vector.tensor_add(zta, psa, xga)
                                st = rtp.tile([128, 12], dt.float32, name="st", tag=f"st{blk % 4}")
                                nc.scalar.activation(st[:], zt[:, 0:12], ACT.Sigmoid)
                                gt = rtp.tile([128, 4], dt.float32, name="gt", tag=f"gt{blk % 4}")
                                nc.scalar.activation(gt[:], zt[:, 12:16], ACT.Tanh)
                                aa = rtp.tile([128, 4], dt.float32, name="aa", tag=f"aa{blk % 4}")
                                nc.vector.tensor_mul(aa[:], st[:, 4:8], c_in[:, blk * 4:(blk + 1) * 4])
                                bb = rtp.tile([128, 4], dt.float32, name="bb", tag=f"bb{blk % 4}")
                                nc.vector.tensor_mul(bb[:], st[:, 0:4], gt[:])
                                nc.vector.tensor_add(c_out[:, blk * 4:(blk + 1) * 4], aa[:], bb[:])
                                tcx = rtp.tile([128, 4], dt.float32, name="tcx", tag=f"tc{blk % 4}")
                                nc.scalar.activation(tcx[:], c_out[:, blk * 4:(blk + 1) * 4], ACT.Tanh)
                                nc.vector.tensor_mul(h_out[:, blk * 4:(blk + 1) * 4], st[:, 8:12], tcx[:])
                                nc.vector.tensor_copy(
                                    hblk[:, blk * 4 * SBLK + s * 4: blk * 4 * SBLK + s * 4 + 4],
                                    h_out[:, blk * 4:(blk + 1) * 4])
                        for blk in range(NBLK):
                            nc.sync.dma_start(
                                hT_out[blk * 128:(blk + 1) * 128, bass.ts(bi, 4 * SBLK)],
                                hblk[:, blk * 4 * SBLK:(blk + 1) * 4 * SBLK])

            # ---------- Ph4: layer-0 recurrence ----------
            recurrence(whh0t, xg0T, h0T)

            # ---------- Ph5: xg1 ----------
            with tc.tile_pool(name="x1", bufs=1) as x1p, \
                 tc.tile_pool(name="x1w", bufs=2) as x1wp, \
                 tc.tile_pool(name="x1ps", bufs=1, space="PSUM") as x1ps:
                h0sb = x1p.tile([128, 8 * F], dt.bfloat16, name="h0sb")
                for k in range(8):
                    nc.sync.dma_start(h0sb[:, k * F:(k + 1) * F], h0T[k * 128:(k + 1) * 128, :])
                for m in range(32):
                    wtile = x1wp.tile([128, 8 * 128], dt.bfloat16, name="w1tile", tag="w1t")
                    for k in range(8):
                        nc.sync.dma_start(
                            wtile[:, k * 128:(k + 1) * 128],
                            wih1t[k * 128:(k + 1) * 128, m * 128:(m + 1) * 128])
                    pss = []
                    for n in range(NCHUNK):
                        ps = x1ps.tile([128, FCH], dt.float32, name="x1psn", tag=f"x1{n}")
                        pss.append(ps)
                    for k in range(8):
                        for n in range(NCHUNK):
                            nc.tensor.matmul(pss[n][:], wtile[:, k * 128:(k + 1) * 128],
                                             h0sb[:, k * F + n * FCH: k * F + n * FCH + FCH],
                                             start=(k == 0), stop=(k == 7))
                    for n in range(NCHUNK):
                        otile = x1wp.tile([128, FCH], dt.float32, name="o1tile", tag="x1o")
                        nc.vector.tensor_scalar_add(otile[:], pss[n][:], b1sb[:, m:m + 1])
                        nc.sync.dma_start(xg1T[m * 128:(m + 1) * 128, n * FCH:(n + 1) * FCH], otile[:])

            # ---------- Ph6: layer-1 recurrence ----------
            recurrence(whh1t, xg1T, h1T)

            # ---------- Ph7: projection ----------
            with tc.tile_pool(name="pj", bufs=1) as pjp, \
                 tc.tile_pool(name="pjw", bufs=2) as pjwp, \
                 tc.tile_pool(name="pjps", bufs=2, space="PSUM") as pjps:
                h1sb = pjp.tile([128, 8 * F], dt.bfloat16, name="h1sb")
                for k in range(8):
                    nc.sync.dma_start(h1sb[:, k * F:(k + 1) * F], h1T[k * 128:(k + 1) * 128, :])
                memTsb2 = pjp.tile([128, 4 * F], dt.float32, name="memTsb2")
                for cb in range(4):
                    nc.sync.dma_start(memTsb2[:, cb * F:(cb + 1) * F], memT_d[cb * 128:(cb + 1) * 128, :])
                wphsb = pjp.tile([128, 8 * M], dt.bfloat16, name="wphsb")
                for k in range(8):
                    nc.sync.dma_start(wphsb[:, k * M:(k + 1) * M], wpt_h[k * 128:(k + 1) * 128, :])
                wpmsb = pjp.tile([128, 4 * M], dt.float32, name="wpmsb")
                for k in range(4):
                    nc.sync.dma_start(wpmsb[:, k * M:(k + 1) * M], wpt_m[k * 128:(k + 1) * 128, :])
                for n in range(NCHUNK):
                    ps = pjps.tile([M, FCH], dt.float32, name="pjpsn", tag=f"pj{n % 4}")
                    for k in range(8):
                        nc.tensor.matmul(ps[:], wphsb[:, k * M:(k + 1) * M],
                                         h1sb[:, k * F + n * FCH: k * F + n * FCH + FCH],
                                         start=(k == 0), stop=False)
                    for cb in range(4):
                        nc.tensor.matmul(ps[:], wpmsb[:, cb * M:(cb + 1) * M],
                                         memTsb2[:, cb * F + n * FCH: cb * F + n * FCH + FCH],
                                         start=False, stop=(cb == 3))
                    otile = pjwp.tile([M, FCH], dt.float32, name="pjo", tag="pjo")
                    nc.vector.tensor_scalar_add(otile[:], ps[:], bpsb[:, 0:1])
                    nc.sync.dma_start(outT[:, n * FCH:(n + 1) * FCH], otile[:])

    nc.finalize()
    return nc


def kernel(memory, y_mels, W1, W2, w_ih0, w_hh0, b_ih0, b_hh0,
           w_ih1, w_hh1, b_ih1, b_hh1, W_proj, b_proj, _trace=False):
    from concourse.bass_utils import run_bass_kernel_spmd

    nc = _build()
    bf16 = ml_dtypes.bfloat16
    ident = np.eye(128, dtype=np.float32)
    w1t = np.ascontiguousarray(W1.T.astype(np.float32))
    w2t = np.ascontiguousarray(W2.T.astype(np.float32))
    wih0t = _arrange_cols(w_ih0.T.astype(np.float32))
    whh0t = _arrange_cols(w_hh0.T.astype(np.float32)).astype(bf16)
    wih1t = _arrange_cols(w_ih1.T.astype(np.float32)).astype(bf16)
    whh1t = _arrange_cols(w_hh1.T.astype(np.float32)).astype(bf16)
    b0 = _arrange_vec((b_ih0 + b_hh0).astype(np.float32)).reshape(1, G4)
    b1 = _arrange_vec((b_ih1 + b_hh1).astype(np.float32)).reshape(1, G4)
    wpt = W_proj.T.astype(np.float32)
    wpt_h = np.ascontiguousarray(wpt[:H]).astype(bf16)
    wpt_m = np.ascontiguousarray(wpt[H:])
    bp = b_proj.astype(np.float32).reshape(1, M)

    in_maps = []
    for c in range(NCORES):
        mem_c = memory[c * BC:(c + 1) * BC]          # [4, 1000, 512]
        y_c = y_mels[c * BC:(c + 1) * BC]            # [4, 1000, 80]
        mem_fc = np.ascontiguousarray(mem_c.transpose(1, 0, 2).reshape(F, A)).astype(np.float32)
        y_fc = np.ascontiguousarray(y_c.transpose(1, 0, 2).reshape(F, M)).astype(np.float32)
        in_maps.append(dict(
            mem_f=mem_fc, y_f=y_fc, ident=ident, w1t=w1t, w2t=w2t,
            wih0t=wih0t, whh0t=whh0t, wih1t=wih1t, whh1t=whh1t,
            b0in=b0, b1in=b1, wpt_h=wpt_h, wpt_m=wpt_m, bpin=bp))

    res = run_bass_kernel_spmd(nc, in_maps, core_ids=list(range(NCORES)), trace=_trace)
    outs = []
    for c in range(NCORES):
        oT = res.results[c]["outT"]                  # [80, 4000]
        outs.append(oT.reshape(M, T, BC).transpose(2, 1, 0))  # [4, 1000, 80]
    full = np.concatenate(outs, axis=0).astype(np.float32)
    if _trace:
        kernel.last_exec_time_ns = res.exec_time_ns
    return full

